# revision 1
# baseline (speedup 1.0000x reference)
"""Causal multi-head attention (B=4, S=2048, D=1024, H=16) on 8 Trainium2 NeuronCores.

Sharding: core c handles batch c//2 and head-group c%2 (8 of 16 heads).
Each core computes qkv projection (f32r matmuls), causal attention
(scores f32r, softmax via ACT exp, PV in bf16 with embedded row-sum
columns), and its 8 heads' slice of the output projection; the host sums
the two half-head partials per batch.

Self-contained: hardcodes shapes; imports concourse from the container's
trn_rl_repo. kernel(**inputs) takes full inputs, returns full output.
"""
import sys

for _p in ("/opt/trn_rl_repo", "/root/.axon_site/_ro/trn_rl_repo"):
    if _p not in sys.path:
        sys.path.append(_p)

import numpy as np

import concourse.bass as bass
import concourse.mybir as mybir
import concourse.tile as tile
from concourse import bacc
from concourse.masks import make_identity

B, S, D, H = 4, 2048, 1024, 16
HD = D // H            # 64
NHL = 8                # heads per core
QB = 1024              # attention q-block
NKC = S // 128         # 16 k-chunks per sequence
dt = mybir.dt
AF = mybir.ActivationFunctionType
P = 128


def build_nc(dbg=False, repeat=1, phases=3):
    nc = bacc.Bacc("TRN2", target_bir_lowering=False, debug=False)

    xs = nc.dram_tensor("xs", [S, D], dt.float32, kind="ExternalInput")
    wqk = nc.dram_tensor("wqk", [P, 8, 8, P], dt.float32, kind="ExternalInput")
    wv = nc.dram_tensor("wv", [P, 8, 4, P], dt.float32, kind="ExternalInput")
    wpj = nc.dram_tensor("wpj", [P, 4, D], dt.float32, kind="ExternalInput")
    out = nc.dram_tensor("out", [S, D], dt.float32, kind="ExternalOutput")
    if dbg:
        d_qt = nc.dram_tensor("d_qt", [P, 4, S], dt.float32, kind="ExternalOutput")
        d_kt = nc.dram_tensor("d_kt", [P, 4, S], dt.float32, kind="ExternalOutput")
        d_v2 = nc.dram_tensor("d_v2", [P, NHL, NKC, 96], dt.bfloat16, kind="ExternalOutput")
        d_yt = nc.dram_tensor("d_yt", [P, 4, S], dt.float32, kind="ExternalOutput")
        d_pv = nc.dram_tensor("d_pv", [P, QB], dt.float32, kind="ExternalOutput")
        d_pt = nc.dram_tensor("d_pt", [P, QB], dt.bfloat16, kind="ExternalOutput")

    from contextlib import ExitStack
    with tile.TileContext(nc) as tc, ExitStack() as _rep:
        if repeat > 1:
            _rep.enter_context(tc.For_i(0, repeat, 1))
        with tc.tile_pool(name="persist", bufs=1) as pp:

            ident = pp.tile([P, P], dt.float32, tag="ident")
            make_identity(nc, ident[:])
            # causal mask tile: 0 where f>=p else -1e30
            maskT = pp.tile([P, P], dt.float32, tag="maskT")
            nc.gpsimd.memset(maskT[:], 0.0)
            nc.gpsimd.affine_select(
                out=maskT[:], in_=maskT[:],
                compare_op=mybir.AluOpType.is_ge, fill=-1e30,
                base=0, pattern=[[1, P]], channel_multiplier=-1)

            QT = pp.tile([P, 4, S], dt.float32r, tag="QT")  # [hd(2-head pair), pair, s]
            KT = pp.tile([P, 4, S], dt.float32r, tag="KT")
            V2 = pp.tile([P, NHL, NKC, 96], dt.bfloat16, tag="V2")  # [k, head, kc, 64 V | 32 ones]
            nc.gpsimd.memset(V2[:, :, :, 64:96], 1.0)
            yT = pp.tile([P, 4, S], dt.float32r, tag="yT")  # [y-dim pair, pair, s]

            # ---------------- Phase Q: x^T, then QKV projections ----------------
            with tc.tile_pool(name="tq1", bufs=1) as tq1, \
                 tc.tile_pool(name="tq", bufs=2) as tq, \
                 tc.tile_pool(name="psA", bufs=4, space="PSUM") as psA:
                for sb in range(4):            # s-blocks of 512
                    xT = tq1.tile([P, 8, 512], dt.float32r, tag="xT")  # [d, dc, s-in-block]
                    for sc in range(4):        # 128-row chunks
                        xn = tq.tile([P, D], dt.float32, tag="xn")
                        nc.sync.dma_start(xn[:], xs[(sb * 4 + sc) * P:(sb * 4 + sc + 1) * P, :])
                        for g in range(2):     # transpose 4 d-chunks per psum tile
                            ptr = psA.tile([P, 512], dt.float32, tag="pmm")
                            for j in range(4):
                                dc = g * 4 + j
                                nc.tensor.transpose(ptr[:, j * P:(j + 1) * P],
                                                    xn[:, dc * P:(dc + 1) * P], ident[:])
                            nc.vector.tensor_copy(
                                xT[:, g * 4:(g + 1) * 4, sc * P:(sc + 1) * P],
                                ptr[:].rearrange("p (j f) -> p j f", j=4))
                    for ch in range(8):        # 4 q-pairs then 4 k-pairs
                        wqkf = tq.tile([P, 8, P], dt.float32, tag="wqkf")
                        nc.sync.dma_start(wqkf[:], wqk[:, :, ch, :])
                        wqkc = tq.tile([P, 8, P], dt.float32r, tag="wqkc")
                        nc.vector.tensor_copy(wqkc[:], wqkf[:])
                        psq = psA.tile([P, 512], dt.float32, tag="pmm")
                        for dc in range(8):
                            nc.tensor.matmul(psq[:], wqkc[:, dc, :], xT[:, dc, :],
                                             start=(dc == 0), stop=(dc == 7))
                        if ch < 4:
                            nc.vector.tensor_copy(QT[:, ch, sb * 512:(sb + 1) * 512], psq[:])
                        else:
                            nc.vector.tensor_copy(KT[:, ch - 4, sb * 512:(sb + 1) * 512], psq[:])
                    for vc in range(4):        # v-pairs -> VT -> transpose -> V natural
                        wvf = tq.tile([P, 8, P], dt.float32, tag="wqkf")
                        nc.sync.dma_start(wvf[:], wv[:, :, vc, :])
                        wvc = tq.tile([P, 8, P], dt.float32r, tag="wqkc")
                        nc.vector.tensor_copy(wvc[:], wvf[:])
                        psv_ = psA.tile([P, 512], dt.float32, tag="pmm")
                        for dc in range(8):
                            nc.tensor.matmul(psv_[:], wvc[:, dc, :], xT[:, dc, :],
                                             start=(dc == 0), stop=(dc == 7))
                        vt = tq.tile([P, 512], dt.float32, tag="vt")
                        nc.vector.tensor_copy(vt[:], psv_[:])
                        for sc in range(4):
                            kc = sb * 4 + sc
                            ptv = psA.tile([P, 512], dt.float32, tag="pmm")
                            nc.tensor.transpose(ptv[:, 0:P], vt[:, sc * P:(sc + 1) * P], ident[:])
                            nc.vector.tensor_copy(V2[:, 2 * vc, kc, 0:64], ptv[:, 0:64])
                            nc.vector.tensor_copy(V2[:, 2 * vc + 1, kc, 0:64], ptv[:, 64:P])

            wpj_r = pp.tile([P, 4, D], dt.float32r, tag="wpj_r")
            with tc.tile_pool(name="wstage", bufs=1) as ws:
                wpjf = ws.tile([P, 4, D], dt.float32, tag="wpjf")
                nc.sync.dma_start(wpjf[:], wpj[:])
                nc.vector.tensor_copy(wpj_r[:], wpjf[:])

            # ---------------- Phase A: causal attention ----------------
            with tc.tile_pool(name="ta", bufs=2) as ta, \
                 tc.tile_pool(name="tpt", bufs=4) as tpt, \
                 tc.tile_pool(name="psS", bufs=2, space="PSUM") as psS, \
                 tc.tile_pool(name="psV", bufs=2, space="PSUM") as psV:
              for h in range(NHL if phases >= 2 else 0):  
                pr = h // 2
                half = slice(0, 64) if h % 2 == 0 else slice(64, P)
                for qb in range(2):
                    nkc = (qb + 1) * 8
                    pv_ps = psV.tile([P, QB], dt.float32, tag="pv")
                    pend = None  # (kc, pT tile, qlo)

                    def emit_pv(kc, pT_t, qlo):
                        q0 = qlo
                        while q0 < QB:
                            q1 = min((q0 // 512 + 1) * 512, QB)  # stay within one PSUM bank
                            nc.tensor.matmul(pv_ps[0:96, q0:q1],
                                             V2[:, h, kc, :], pT_t[:, q0:q1],
                                             start=(kc == 0), stop=(kc == nkc - 1),
                                             skip_group_check=True)
                            q0 = q1

                    for kc in range(nkc):
                        qlo = max(0, kc * P - qb * QB)
                        sc_ps = psS.tile([P, QB], dt.float32, tag="sc")
                        q0 = qlo
                        while q0 < QB:
                            q1 = min((q0 // 512 + 1) * 512, QB)  # stay within one PSUM bank
                            nc.tensor.matmul(sc_ps[:, q0:q1],
                                             KT[half, pr, kc * P:(kc + 1) * P],
                                             QT[half, pr, qb * QB + q0:qb * QB + q1],
                                             start=True, stop=True)
                            q0 = q1
                        if kc * P >= qb * QB:  # diagonal chunk: -1e30 on k>q corner
                            nc.vector.tensor_tensor(sc_ps[:, qlo:qlo + P],
                                                    sc_ps[:, qlo:qlo + P], maskT[:],
                                                    mybir.AluOpType.add)
                        pT_t = tpt.tile([P, QB], dt.bfloat16, tag="pT")
                        nc.scalar.activation(pT_t[:, qlo:QB], sc_ps[:, qlo:QB],
                                             AF.Exp, scale=0.125)
                        if pend is not None:
                            emit_pv(*pend)
                        pend = (kc, pT_t, qlo)
                    emit_pv(*pend)
                    if dbg and h == 0 and qb == 0:
                        dpv = ta.tile([P, QB], dt.float32, tag="dpv")
                        nc.vector.tensor_copy(dpv[:], pv_ps[:])
                        nc.sync.dma_start(d_pv[:], dpv[:])
                        nc.sync.dma_start(d_pt[:], pend[1][:])

                    # normalization: r = exp(-ln(sums)); sums dup on rows 64:96
                    tln = ta.tile([P, QB], dt.float32, tag="tln")
                    nc.scalar.activation(tln[64:96, :], pv_ps[64:96, :], AF.Ln)
                    trc = ta.tile([P, QB], dt.float32, tag="trc")
                    nc.scalar.activation(trc[64:96, :], tln[64:96, :], AF.Exp, scale=-1.0)
                    rsh = ta.tile([64, QB], dt.float32, tag="rsh")
                    nc.sync.dma_start(rsh[0:32, :], trc[64:96, :])
                    nc.sync.dma_start(rsh[32:64, :], trc[64:96, :])
                    if h % 2 == 0:
                        nc.vector.tensor_tensor(yT[0:64, pr, qb * QB:(qb + 1) * QB],
                                                pv_ps[0:64, :], rsh[:],
                                                mybir.AluOpType.mult)
                    else:
                        ytmp = ta.tile([64, QB], dt.float32r, tag="ytmp")
                        nc.vector.tensor_tensor(ytmp[:], pv_ps[0:64, :], rsh[:],
                                                mybir.AluOpType.mult)
                        nc.sync.dma_start(yT[64:P, pr, qb * QB:(qb + 1) * QB], ytmp[:])

            if dbg:
                nc.sync.dma_start(d_qt[:], QT[:].bitcast(dt.float32))
                nc.sync.dma_start(d_kt[:], KT[:].bitcast(dt.float32))
                nc.sync.dma_start(d_v2[:], V2[:])
                nc.sync.dma_start(d_yt[:], yT[:].bitcast(dt.float32))

            # ---------------- Phase P: output projection ----------------
            if phases < 3:
                # keep results live: dump QT/KT/V2/yT slices into out
                nc.sync.dma_start(out[0:P, 0:D], QT[:, 0, 0:D].bitcast(dt.float32).unsqueeze(1))
                nc.sync.dma_start(out[P:2 * P, 0:D], KT[:, 1, 0:D].bitcast(dt.float32).unsqueeze(1))
                if phases >= 2:
                    nc.sync.dma_start(out[2 * P:3 * P, 0:D], yT[:, 2, 0:D].bitcast(dt.float32).unsqueeze(1))
                so0 = pp.tile([P, D], dt.float32, tag="so0")
                nc.vector.tensor_copy(so0[:], V2[:, 0, :, :].rearrange("p a b -> p (a b)")[:, 0:D])
                nc.sync.dma_start(out[3 * P:4 * P, 0:D], so0[:])
            with tc.tile_pool(name="tp", bufs=4) as tp, \
                 tc.tile_pool(name="psP", bufs=6, space="PSUM") as psP:
                for sc in range(16 if phases >= 3 else 0):
                    for oc in range(2):
                        pps = psP.tile([P, 512], dt.float32, tag="pp")
                        for pc in range(4):
                            nc.tensor.matmul(pps[:], yT[:, pc, sc * P:(sc + 1) * P],
                                             wpj_r[:, pc, oc * 512:(oc + 1) * 512],
                                             start=(pc == 0), stop=(pc == 3))
                        so = tp.tile([P, 512], dt.float32, tag="so")
                        nc.vector.tensor_copy(so[:], pps[:])
                        nc.sync.dma_start(out[sc * P:(sc + 1) * P, oc * 512:(oc + 1) * 512], so[:])

    nc.compile()
    return nc


def prepare_inputs(x, Wqkv, Wproj):
    """Pack per-core inputs. Core c: batch c//2, heads (c%2)*8 .. +8."""
    x = np.asarray(x, dtype=np.float32)
    Wqkv = np.asarray(Wqkv, dtype=np.float32)
    Wproj = np.asarray(Wproj, dtype=np.float32)
    in_maps = []
    for c in range(8):
        b, g = c // 2, c % 2
        hg = g * NHL
        wqk = np.empty((P, 8, 8, P), dtype=np.float32)
        wv = np.empty((P, 8, 4, P), dtype=np.float32)
        # Wqkv rows d = dc*128 + p
        Wq = Wqkv[:, :D].reshape(8, P, H, HD)       # [dc, p, head, hd]
        Wk = Wqkv[:, D:2 * D].reshape(8, P, H, HD)
        Wv_ = Wqkv[:, 2 * D:].reshape(8, P, H, HD)
        for ch in range(4):
            wqk[:, :, ch, 0:64] = Wq[:, :, hg + 2 * ch, :].transpose(1, 0, 2)
            wqk[:, :, ch, 64:P] = Wq[:, :, hg + 2 * ch + 1, :].transpose(1, 0, 2)
            wqk[:, :, ch + 4, 0:64] = Wk[:, :, hg + 2 * ch, :].transpose(1, 0, 2)
            wqk[:, :, ch + 4, 64:P] = Wk[:, :, hg + 2 * ch + 1, :].transpose(1, 0, 2)
            wv[:, :, ch, 0:64] = Wv_[:, :, hg + 2 * ch, :].transpose(1, 0, 2)
            wv[:, :, ch, 64:P] = Wv_[:, :, hg + 2 * ch + 1, :].transpose(1, 0, 2)
        wpj = np.empty((P, 4, D), dtype=np.float32)
        for pc in range(4):
            wpj[0:64, pc, :] = Wproj[HD * (hg + 2 * pc):HD * (hg + 2 * pc) + HD, :]
            wpj[64:P, pc, :] = Wproj[HD * (hg + 2 * pc + 1):HD * (hg + 2 * pc + 1) + HD, :]
        in_maps.append({
            "xs": np.ascontiguousarray(x[b]),
            "wqk": wqk, "wv": wv, "wpj": wpj,
        })
    return in_maps


def combine_outputs(results):
    out = np.empty((B, S, D), dtype=np.float32)
    for b in range(B):
        out[b] = results[2 * b]["out"] + results[2 * b + 1]["out"]
    return out


_NC_CACHE = None


def get_nc():
    global _NC_CACHE
    if _NC_CACHE is None:
        _NC_CACHE = build_nc()
    return _NC_CACHE


def kernel(x, Wqkv, Wproj):
    from concourse.bass_utils import run_bass_kernel_spmd
    nc = get_nc()
    in_maps = prepare_inputs(x, Wqkv, Wproj)
    res = run_bass_kernel_spmd(nc, in_maps, core_ids=list(range(8)))
    return combine_outputs(res.results)


if __name__ == "__main__":
    rng = np.random.default_rng(0)
    x = rng.standard_normal((B, S, D), dtype=np.float32)
    Wqkv = (rng.standard_normal((D, 3 * D), dtype=np.float32) / np.sqrt(D)).astype(np.float32)
    Wproj = (rng.standard_normal((D, D), dtype=np.float32) / np.sqrt(D)).astype(np.float32)
    y = kernel(x, Wqkv, Wproj)
    print("ok", y.shape, float(np.abs(y).max()))



# revision 3
# speedup vs baseline: 1.5582x; 1.5582x over previous
"""Causal multi-head attention (B=4, S=2048, D=1024, H=16) on 8 Trainium2 NeuronCores.

Sharding: core c handles batch c//2 and head-group c%2 (8 of 16 heads).
Each core computes its 8 heads' qkv projection, causal attention, and its
slice of the output projection; the host sums the two half-head partials
per batch.

All matmul operands are bf16 (host-prepped): x arrives pre-transposed as
xt[d, s], weights in matmul-ready layouts with the 1/sqrt(hd) softmax
scale folded into Wq. V is produced directly in natural [k, vdim] layout
by using x^T chunks as the stationary operand, so no on-chip transposes
are needed. Softmax row-sums ride along the PV matmul as 32 ones-columns
in the stationary operand; the reciprocal runs on DVE.

Self-contained: hardcodes shapes; imports concourse from the container's
trn_rl_repo. kernel(**inputs) takes full inputs, returns full output.
"""
import sys

for _p in ("/opt/trn_rl_repo", "/root/.axon_site/_ro/trn_rl_repo"):
    if _p not in sys.path:
        sys.path.append(_p)

import numpy as np

import concourse.bass as bass
import concourse.mybir as mybir
import concourse.tile as tile
from concourse import bacc

B, S, D, H = 4, 2048, 1024, 16
HD = D // H            # 64
NHL = 8                # heads per core
QB = 1024              # attention q-block
NKC = S // 128         # 16 k-chunks per sequence
dt = mybir.dt
AF = mybir.ActivationFunctionType
P = 128


def build_nc(repeat=1):
    nc = bacc.Bacc("TRN2", target_bir_lowering=False, debug=False)

    # xt[p, dc, s] = x[s, dc*128 + p]  (pre-transposed, bf16)
    xt = nc.dram_tensor("xt", [P, 8, S], dt.bfloat16, kind="ExternalInput")
    # wqk[p, ch, dc, j]: ch 0-3 q-pairs, 4-7 k-pairs; j = hd of head pair
    wqk = nc.dram_tensor("wqk", [P, 8, 8, P], dt.bfloat16, kind="ExternalInput")
    # wv[p, dc, h*64+j] = Wv[dc*128+p, (hg+h)*64+j]
    wv = nc.dram_tensor("wv", [P, 8, 512], dt.bfloat16, kind="ExternalInput")
    # wpj[j, pc, :]: rows = vdim of head pair pc
    wpj = nc.dram_tensor("wpj", [P, 4, D], dt.bfloat16, kind="ExternalInput")
    out = nc.dram_tensor("out", [S, D], dt.float32, kind="ExternalOutput")

    from contextlib import ExitStack
    with tile.TileContext(nc) as tc, ExitStack() as _rep:
        if repeat > 1:
            _rep.enter_context(tc.For_i(0, repeat, 1))
        with tc.tile_pool(name="persist", bufs=1) as pp:

            # causal mask tile: 0 where f>=p else -1e30
            maskT = pp.tile([P, P], dt.float32, tag="maskT")
            nc.gpsimd.memset(maskT[:], 0.0)
            nc.gpsimd.affine_select(
                out=maskT[:], in_=maskT[:],
                compare_op=mybir.AluOpType.is_ge, fill=-1e30,
                base=0, pattern=[[1, P]], channel_multiplier=-1)

            xT = pp.tile([P, 8, S], dt.bfloat16, tag="xT")
            wqks = pp.tile([P, 8, 8, P], dt.bfloat16, tag="wqks")
            wvs = pp.tile([P, 8, 512], dt.bfloat16, tag="wvs")
            wpjs = pp.tile([P, 4, D], dt.bfloat16, tag="wpjs")

            QT = pp.tile([P, 4, S], dt.bfloat16, tag="QT")  # [hd pair, pair, s]
            KT = pp.tile([P, 4, S], dt.bfloat16, tag="KT")
            V2 = pp.tile([P, NHL, NKC, 96], dt.bfloat16, tag="V2")  # [k, h, kc, 64 V | 32 ones]
            nc.gpsimd.memset(V2[:, :, :, 64:96], 1.0)
            yT = pp.tile([P, 4, S], dt.bfloat16, tag="yT")  # [vdim pair, pair, s]

            # stage inputs (weights per-chunk so the first matmuls start early)
            for ch in range(8):
                nc.sync.dma_start(wqks[:, ch], wqk[:, ch])
            for sb in range(4):
                nc.sync.dma_start(xT[:, :, sb * 512:(sb + 1) * 512],
                                  xt[:, :, sb * 512:(sb + 1) * 512])
            nc.sync.dma_start(wvs[:], wv[:])
            nc.sync.dma_start(wpjs[:], wpj[:])

            # ---------------- Phase Q: QKV projections ----------------
            with tc.tile_pool(name="psA", bufs=4, space="PSUM") as psA:
                for ch in range(8):        # 4 q-pairs then 4 k-pairs
                    for sb in range(4):    # s-blocks of 512
                        psq = psA.tile([P, 512], dt.float32, tag="pmm")
                        for dc in range(8):
                            nc.tensor.matmul(psq[:], wqks[:, ch, dc, :],
                                             xT[:, dc, sb * 512:(sb + 1) * 512],
                                             start=(dc == 0), stop=(dc == 7))
                        dst = QT if ch < 4 else KT
                        nc.scalar.copy(dst[:, ch % 4, sb * 512:(sb + 1) * 512], psq[:])
                for sc in range(16):       # V directly in natural [k, vdim] layout
                    psv = psA.tile([P, 512], dt.float32, tag="pmm")
                    for dc in range(8):
                        nc.tensor.matmul(psv[:], xT[:, dc, sc * P:(sc + 1) * P],
                                         wvs[:, dc, :],
                                         start=(dc == 0), stop=(dc == 7))
                    nc.vector.tensor_copy(
                        V2[:, :, sc, 0:64],
                        psv[:].rearrange("p (h f) -> p h f", h=8))

            # ---------------- Phase A: causal attention ----------------
            with tc.tile_pool(name="ta", bufs=2) as ta, \
                 tc.tile_pool(name="tpt", bufs=4) as tpt, \
                 tc.tile_pool(name="psS", bufs=2, space="PSUM") as psS, \
                 tc.tile_pool(name="psV", bufs=2, space="PSUM") as psV:
              for h in range(NHL):
                pr = h // 2
                half = slice(0, 64) if h % 2 == 0 else slice(64, P)
                for qb in range(2):
                    nkc = (qb + 1) * 8
                    pv_ps = psV.tile([P, QB], dt.float32, tag="pv")
                    pend = None  # (kc, pT tile, qlo)

                    def emit_pv(kc, pT_t, qlo):
                        q0 = qlo
                        while q0 < QB:
                            q1 = min((q0 // 512 + 1) * 512, QB)  # stay within one PSUM bank
                            nc.tensor.matmul(pv_ps[0:96, q0:q1],
                                             V2[:, h, kc, :], pT_t[:, q0:q1],
                                             start=(kc == 0), stop=(kc == nkc - 1),
                                             skip_group_check=True)
                            q0 = q1

                    for kc in range(nkc):
                        qlo = max(0, kc * P - qb * QB)
                        sc_ps = psS.tile([P, QB], dt.float32, tag="sc")
                        q0 = qlo
                        while q0 < QB:
                            q1 = min((q0 // 512 + 1) * 512, QB)  # stay within one PSUM bank
                            nc.tensor.matmul(sc_ps[:, q0:q1],
                                             KT[half, pr, kc * P:(kc + 1) * P],
                                             QT[half, pr, qb * QB + q0:qb * QB + q1],
                                             start=True, stop=True)
                            q0 = q1
                        if kc * P >= qb * QB:  # diagonal chunk: -1e30 on k>q corner
                            nc.vector.tensor_tensor(sc_ps[:, qlo:qlo + P],
                                                    sc_ps[:, qlo:qlo + P], maskT[:],
                                                    mybir.AluOpType.add)
                        pT_t = tpt.tile([P, QB], dt.bfloat16, tag="pT")
                        nc.scalar.activation(pT_t[:, qlo:QB], sc_ps[:, qlo:QB], AF.Exp)
                        if pend is not None:
                            emit_pv(*pend)
                        pend = (kc, pT_t, qlo)
                    emit_pv(*pend)

                    # normalization: r = 1/sums; sums dup on rows 64:96
                    trc = ta.tile([P, QB], dt.float32, tag="trc")
                    nc.vector.reciprocal(trc[64:96, :], pv_ps[64:96, :])
                    rsh = ta.tile([64, QB], dt.float32, tag="rsh")
                    nc.sync.dma_start(rsh[0:32, :], trc[64:96, :])
                    nc.sync.dma_start(rsh[32:64, :], trc[64:96, :])
                    if h % 2 == 0:
                        nc.vector.tensor_tensor(yT[0:64, pr, qb * QB:(qb + 1) * QB],
                                                pv_ps[0:64, :], rsh[:],
                                                mybir.AluOpType.mult)
                    else:
                        ytmp = ta.tile([64, QB], dt.bfloat16, tag="ytmp")
                        nc.vector.tensor_tensor(ytmp[:], pv_ps[0:64, :], rsh[:],
                                                mybir.AluOpType.mult)
                        nc.sync.dma_start(yT[64:P, pr, qb * QB:(qb + 1) * QB], ytmp[:])

            # ---------------- Phase P: output projection ----------------
            with tc.tile_pool(name="tp", bufs=4) as tp, \
                 tc.tile_pool(name="psP", bufs=4, space="PSUM") as psP:
                for sc in range(16):
                    for oc in range(2):
                        pps = psP.tile([P, 512], dt.float32, tag="pp")
                        for pc in range(4):
                            nc.tensor.matmul(pps[:], yT[:, pc, sc * P:(sc + 1) * P],
                                             wpjs[:, pc, oc * 512:(oc + 1) * 512],
                                             start=(pc == 0), stop=(pc == 3))
                        so = tp.tile([P, 512], dt.float32, tag="so")
                        nc.scalar.copy(so[:], pps[:])
                        nc.sync.dma_start(out[sc * P:(sc + 1) * P, oc * 512:(oc + 1) * 512], so[:])

    nc.compile()
    return nc


def prepare_inputs(x, Wqkv, Wproj):
    """Pack per-core bf16 inputs. Core c: batch c//2, heads (c%2)*8 .. +8."""
    from ml_dtypes import bfloat16
    x = np.asarray(x, dtype=np.float32)
    Wqkv = np.asarray(Wqkv, dtype=np.float32)
    Wproj = np.asarray(Wproj, dtype=np.float32)
    scale = 1.0 / np.sqrt(HD)
    # Wqkv rows d = dc*128 + p
    Wq = (Wqkv[:, :D] * scale).reshape(8, P, H, HD)  # [dc, p, head, hd]
    Wk = Wqkv[:, D:2 * D].reshape(8, P, H, HD)
    Wv_ = Wqkv[:, 2 * D:].reshape(8, P, H, HD)
    in_maps = []
    for c in range(8):
        b, g = c // 2, c % 2
        hg = g * NHL
        wqk = np.empty((P, 8, 8, P), dtype=np.float32)
        for ch in range(4):
            wqk[:, ch, :, 0:64] = Wq[:, :, hg + 2 * ch, :].transpose(1, 0, 2)
            wqk[:, ch, :, 64:P] = Wq[:, :, hg + 2 * ch + 1, :].transpose(1, 0, 2)
            wqk[:, ch + 4, :, 0:64] = Wk[:, :, hg + 2 * ch, :].transpose(1, 0, 2)
            wqk[:, ch + 4, :, 64:P] = Wk[:, :, hg + 2 * ch + 1, :].transpose(1, 0, 2)
        # wv[p, dc, h*64+j] = Wv[dc*128+p, (hg+h)*64+j]
        wv = Wv_[:, :, hg:hg + NHL, :].reshape(8, P, NHL * HD).transpose(1, 0, 2)
        wpj = np.empty((P, 4, D), dtype=np.float32)
        for pc in range(4):
            wpj[0:64, pc, :] = Wproj[HD * (hg + 2 * pc):HD * (hg + 2 * pc) + HD, :]
            wpj[64:P, pc, :] = Wproj[HD * (hg + 2 * pc + 1):HD * (hg + 2 * pc + 1) + HD, :]
        # xt[p, dc, s] = x[b, s, dc*128+p]
        xt = np.ascontiguousarray(x[b].T.reshape(8, P, S).transpose(1, 0, 2))
        in_maps.append({
            "xt": xt.astype(bfloat16),
            "wqk": np.ascontiguousarray(wqk).astype(bfloat16),
            "wv": np.ascontiguousarray(wv).astype(bfloat16),
            "wpj": wpj.astype(bfloat16),
        })
    return in_maps


def combine_outputs(results):
    out = np.empty((B, S, D), dtype=np.float32)
    for b in range(B):
        out[b] = results[2 * b]["out"] + results[2 * b + 1]["out"]
    return out


_NC_CACHE = None


def get_nc():
    global _NC_CACHE
    if _NC_CACHE is None:
        _NC_CACHE = build_nc()
    return _NC_CACHE


def kernel(x, Wqkv, Wproj):
    from concourse.bass_utils import run_bass_kernel_spmd
    nc = get_nc()
    in_maps = prepare_inputs(x, Wqkv, Wproj)
    res = run_bass_kernel_spmd(nc, in_maps, core_ids=list(range(8)))
    return combine_outputs(res.results)


if __name__ == "__main__":
    rng = np.random.default_rng(0)
    x = rng.standard_normal((B, S, D), dtype=np.float32)
    Wqkv = (rng.standard_normal((D, 3 * D), dtype=np.float32) / np.sqrt(D)).astype(np.float32)
    Wproj = (rng.standard_normal((D, D), dtype=np.float32) / np.sqrt(D)).astype(np.float32)
    y = kernel(x, Wqkv, Wproj)
    print("ok", y.shape, float(np.abs(y).max()))
